# revision 1
# baseline (speedup 1.0000x reference)
"""AngProtoLoss (stable) distributed Bass kernel for 8 TRN2 NeuronCores.

Problem (reference):
    dvecs: (4096, 16, 512) f32
    centroids = mean(dvecs, axis=1)                  # (N, D)
    u = dvecs[:, -1, :]                              # (N, D)
    cos = clip(cos_sim(u, centroids), min=1e-6)      # (N, N)
    logits = cos * w + b
    loss = -mean(diag(log_softmax(logits)))
        = mean_i [ logsumexp_k(w*clip(cos_ik)) - w*clip(cos_ii) ]   (b cancels)

Sharding: data-parallel over speakers N. Each core gets 512 speakers (4
chunks of 128), computes local normalized centroids (bf16 tree sum ->
rsqrt-normalize), transposes them on the TensorE, all-gathers them in fp8
(one allgather per chunk, pipelined against the load/centroid phase), then
computes its 512 rows of the cos matrix in bf16 x fp8 matmuls, applies
clip+exp (with ScalarE accumulate) for the log-sum-exp, and the local
diagonal terms. Device outputs per-row exp-sums and diagonal cos; the host
unshard does rows = log(s) - w*clip(diag) and means over N (b cancels in
log-softmax exactly).

Schedule notes (engine queues are FIFO):
 - gpsimd queue holds only the bounce writes + collectives so each allgather
   triggers as soon as its bounce is written (never stuck behind big loads).
 - X loads and gathered reads ride the sync HWDGE ring in data-ready order.
 - explicit add_dep edges keep phase-C matmuls/epilogue behind all phase-B
   work on PE/DVE/ACT queues, so a late chunk's transposes are never stalled
   behind ops waiting on a gather.
 - bounce layout is [128 d-rows x 512B (t,i)-cols] so every DMA touching
   HBM moves >=512B contiguous runs (small descriptors starve during
   collectives).
"""

import os
import sys

for _p in ("/opt/trn_rl_repo",):
    if os.path.isdir(_p) and _p not in sys.path:
        sys.path.append(_p)

import numpy as np

import concourse.bass as bass
import concourse.tile as tile
from concourse import bacc, mybir
from concourse.bass_utils import run_bass_kernel_spmd
from concourse.masks import make_identity

N_CORES = 8
N, M, D = 4096, 16, 512
P = 128                     # partitions
LOCAL = N // N_CORES        # 512 speakers per core
NCHUNK = LOCAL // P         # 4 chunks of 128 speakers
NT = D // P                 # 4 d-tiles
EPS = 1e-6

F32 = mybir.dt.float32
BF16 = mybir.dt.bfloat16
FP8 = mybir.dt.float8e4
AF = mybir.ActivationFunctionType


def build_program(w_val: float):
    nc = bacc.Bacc("TRN2", target_bir_lowering=False, debug=False,
                   num_devices=N_CORES)
    dvecs = nc.dram_tensor("dvecs", [LOCAL, M, D], F32, kind="ExternalInput").ap()
    out = nc.dram_tensor("out", [2, LOCAL], F32, kind="ExternalOutput").ap()

    with tile.TileContext(nc) as tc:
        _build(nc, tc, dvecs, out, w_val)
    nc.compile()
    return nc


def _build(nc, tc, dvecs, out, w_val):
    from contextlib import ExitStack
    ctx = ExitStack()
    with ctx:
        singles = ctx.enter_context(tc.tile_pool(name="singles", bufs=1))
        xpool = ctx.enter_context(tc.tile_pool(name="xpool", bufs=2))
        tree = ctx.enter_context(tc.tile_pool(name="tree", bufs=2))
        cpool = ctx.enter_context(tc.tile_pool(name="cpool", bufs=2))
        stats = ctx.enter_context(tc.tile_pool(name="stats", bufs=4))
        gpool = ctx.enter_context(tc.tile_pool(name="gpool", bufs=1))
        epool = ctx.enter_context(tc.tile_pool(name="epool", bufs=4))
        tpsum = ctx.enter_context(tc.tile_pool(name="tpsum", bufs=2, space="PSUM"))
        mpsum = ctx.enter_context(tc.tile_pool(name="mpsum", bufs=3, space="PSUM"))
        dram = ctx.enter_context(tc.tile_pool(name="dram", bufs=1, space="DRAM"))

        ident = singles.tile([P, P], F32)
        make_identity(nc, ident)

        # persistent across the whole kernel
        uT = singles.tile([P, NT, LOCAL], BF16)          # u^T: [d_in_tile, t, i]
        s_acc = singles.tile([P, NCHUNK], F32)           # sum_k exp(w*clip(cos))
        diag_all = singles.tile([P, NCHUNK], F32)        # diag cos, per q
        nc.vector.memset(s_acc, 0.0)

        # ---------- phase A: loads first (sync ring order) ----------
        xs = []
        for r in range(NCHUNK):
            x = xpool.tile([P, M, D], F32, name=f"x{r}", tag="x")
            nc.sync.dma_start(out=x, in_=dvecs[r * P:(r + 1) * P, :, :])
            xs.append(x)

        # ---------- phase B: per-chunk centroid pipeline + allgather ----------
        # One allgather per chunk (grouping chunks into fewer, bigger AGs
        # measured slower: the 1MB fp8 AG falls into the slow RDH regime).
        GROUPS = [[0], [1], [2], [3]]
        chunk_group = {}
        for gi, grp in enumerate(GROUPS):
            for slot, rr in enumerate(grp):
                chunk_group[rr] = (gi, slot)
        bounces = [None] * len(GROUPS)
        cc_insts = []
        gath = []
        last_transpose = [None]
        last_dve_b = [None]
        last_act_b = [None]
        # ssq/scale slots for all chunks: [:, r, 0] = centroid, [:, r, 1] = u.
        # Norm transcendentals (Ln then Exp) run batched per chunk PAIR so the
        # ACT table set switches ~4x per kernel instead of 14x -- each switch
        # is a 1.5us TDRAM DMA that lands inside the collective windows.
        ssq_all = singles.tile([P, NCHUNK, 2], F32)
        scales_all = singles.tile([P, NCHUNK, 2], F32)
        csums = []
        u_saves = []
        for r in range(NCHUNK):
            x = xs[r]
            # centroid sum over m: first level casts f32 -> bf16
            t1 = tree.tile([P, M // 2, D], BF16, name=f"t1_{r}", tag="t1")
            for j in range(M // 2):
                nc.vector.tensor_add(t1[:, j, :], x[:, 2 * j, :], x[:, 2 * j + 1, :])
            t2 = tree.tile([P, M // 4, D], BF16, name=f"t2_{r}", tag="t2")
            for j in range(M // 4):
                nc.vector.tensor_add(t2[:, j, :], t1[:, 2 * j, :], t1[:, 2 * j + 1, :])
            t3 = tree.tile([P, M // 8, D], BF16, name=f"t3_{r}", tag="t3")
            for j in range(M // 8):
                nc.vector.tensor_add(t3[:, j, :], t2[:, 2 * j, :], t2[:, 2 * j + 1, :])
            csum = cpool.tile([P, D], BF16, name=f"csum{r}", tag="csum")
            nc.vector.tensor_add(csum, t3[:, 0, :], t3[:, 1, :])
            csums.append(csum)

            # save the last utterance (frees the big X tile early)
            u_save = cpool.tile([P, D], BF16, name=f"usave{r}", tag="usave")
            nc.vector.tensor_copy(u_save, x[:, M - 1, :])
            u_saves.append(u_save)

            sq_scr = cpool.tile([P, D], BF16, name=f"sqscr{r}", tag="sqscr")
            nc.vector.tensor_mul(sq_scr, csum, csum)
            nc.vector.tensor_reduce(ssq_all[:, r, 0:1], sq_scr,
                                    axis=mybir.AxisListType.X,
                                    op=mybir.AluOpType.add)
            nc.vector.tensor_mul(sq_scr, x[:, M - 1, :], x[:, M - 1, :])
            nc.vector.tensor_reduce(ssq_all[:, r, 1:2], sq_scr,
                                    axis=mybir.AxisListType.X,
                                    op=mybir.AluOpType.add)

            if r % 2 == 0:
                continue
            # ---- batched norms + downstream for the pair (r-1, r) ----
            p0 = r - 1
            ln_scr = stats.tile([P, 2, 2], F32, name=f"ln{r}", tag="ln")
            nc.scalar.activation(ln_scr, ssq_all[:, p0:r + 1, :], AF.Ln)
            nc.scalar.activation(scales_all[:, p0:r + 1, :], ln_scr,
                                 AF.Exp, scale=-0.5)
            for rr in (p0, r):
                # normalize (f32 out: PSUM->SBUF copies ride ScalarE)
                chat = cpool.tile([P, D], F32, name=f"chat{rr}", tag="chat")
                uhat = cpool.tile([P, D], F32, name=f"uhat{rr}", tag="uhat")
                nc.vector.tensor_scalar_mul(chat, csums[rr],
                                            scales_all[:, rr, 0:1])
                nc.vector.tensor_scalar_mul(uhat, u_saves[rr],
                                            scales_all[:, rr, 1:2])

                # diagonal cos (local)
                dg_scr = cpool.tile([P, D], F32, name=f"dgscr{rr}", tag="dgscr")
                nc.vector.tensor_mul(dg_scr, chat, uhat)
                rd = nc.vector.tensor_reduce(diag_all[:, rr:rr + 1], dg_scr,
                                             axis=mybir.AxisListType.X,
                                             op=mybir.AluOpType.add)
                last_dve_b[0] = rd.ins

                # transposes on PE (f32 -> f32 psum), cast to fp8/bf16 on ACT
                cT = cpool.tile([P, NT, P], FP8, name=f"cT{rr}", tag="cT")
                for t in range(NT):
                    pt = tpsum.tile([P, P], F32, name=f"ptc{rr}_{t}", tag="pt")
                    ti = nc.tensor.transpose(pt, chat[:, t * P:(t + 1) * P],
                                             ident)
                    last_transpose[0] = ti.ins
                    nc.scalar.copy(cT[:, t, :], pt)
                    pu = tpsum.tile([P, P], F32, name=f"ptu{rr}_{t}", tag="pt")
                    ti = nc.tensor.transpose(pu, uhat[:, t * P:(t + 1) * P],
                                             ident)
                    last_transpose[0] = ti.ins
                    cp = nc.scalar.copy(uT[:, t, rr * P:(rr + 1) * P], pu)
                    last_act_b[0] = cp.ins

                # bounce write (fp8) on the gpsimd SWDGE ring (otherwise
                # empty) so it is not FIFO-serialized behind the X loads.
                gi, slot = chunk_group[rr]
                L = len(GROUPS[gi])
                bounces[gi] = bounces[gi] if bounces[gi] is not None else \
                    dram.tile([L * P, NT * P], FP8, name=f"bounce_g{gi}")
                nc.gpsimd.dma_start(
                    out=bounces[gi][slot * P:(slot + 1) * P, :],
                    in_=cT.rearrange("p t i -> p (t i)"))
                if slot == L - 1:
                    g = dram.tile([N_CORES * L * P, NT * P], FP8,
                                  name=f"gath{gi}", addr_space="Shared")
                    cc = nc.gpsimd.collective_compute(
                        "AllGather", mybir.AluOpType.bypass,
                        replica_groups=[list(range(N_CORES))],
                        ins=[bounces[gi].opt()], outs=[g.opt()],
                    )
                    cc_insts.append(cc.ins)
                    gath.append(g)

        # diag rows are complete after phase B: ship them now, off the tail
        nc.sync.dma_start(out=out[1].rearrange("(q p) -> p q", p=P),
                          in_=diag_all)

        # ---------- phase C: gathered reads + matmuls + epilogue ----------
        # Ordering guards: phase-C work on DVE/ACT must sit behind all
        # phase-B work on those queues, so late chunks are never stalled
        # behind epilogue ops waiting on a gather.
        last_dve = [last_dve_b[0]]
        last_act = [last_act_b[0]]
        for gi, grp in enumerate(GROUPS):
            L = len(grp)
            g_sb = gpool.tile([P, N_CORES * L, NT, P], FP8, name=f"gsb{gi}",
                              tag=f"gsb{gi}")
            nc.sync.dma_start(
                out=g_sb,
                in_=gath[gi].rearrange("(c p) f -> p c f", p=P).rearrange(
                    "p c (t i) -> p c t i", t=NT))
            for q in range(NCHUNK):
                for slot in range(L):
                    ps = mpsum.tile([P, 2, N_CORES // 2 * P], F32,
                                    name=f"ps{gi}_{q}_{slot}", tag="ps")
                    for h in range(2):
                        for t in range(NT):
                            # rhs: ranks c in [4h,4h+4), chunk slot, d-tile t
                            cs = 4 * h * L + slot
                            rhs = g_sb[:, cs:cs + 3 * L + 1:L, t, :]
                            mm = nc.tensor.matmul(
                                ps[:, h, :],
                                uT[:, t, q * P:(q + 1) * P],
                                rhs,
                                start=(t == 0), stop=(t == NT - 1),
                            )
                            # keep every matmul behind all transposes in the
                            # PE queue so late-chunk transposes are never
                            # stalled by matmuls waiting on a gather.
                            if h == 0 and t == 0:
                                tile.add_dep_helper(
                                    mm.ins, last_transpose[0], sync=True,
                                    reason="PE: transposes before matmuls")
                    # epilogue: y = max(cos, eps); s += sum_k exp(w*y)
                    y = epool.tile([P, 2 * (N_CORES // 2) * P], BF16,
                                   name=f"y{gi}_{q}_{slot}", tag="y")
                    mx = nc.vector.tensor_scalar_max(
                        y, ps.rearrange("p a b -> p (a b)"), EPS)
                    e_scr = epool.tile([P, 2 * (N_CORES // 2) * P], BF16,
                                       name=f"escr{gi}_{q}_{slot}", tag="escr")
                    s_part = stats.tile([P, 1], F32, name=f"sp{gi}_{q}_{slot}",
                                        tag="sp")
                    ex = nc.scalar.activation(e_scr, y, AF.Exp, scale=w_val,
                                              accum_out=s_part)
                    nc.vector.tensor_add(s_acc[:, q:q + 1], s_acc[:, q:q + 1],
                                         s_part)

        # ---------- finals: ship s (exp-sums); host does the log.
        # (diag was already shipped right after phase B, off the tail.)
        nc.sync.dma_start(out=out[0].rearrange("(q p) -> p q", p=P), in_=s_acc)


_CACHE = {}


def kernel(dvecs, w, b):
    w_val = float(np.asarray(w))
    key = w_val
    if key not in _CACHE:
        _CACHE[key] = build_program(w_val)
    nc = _CACHE[key]
    dvecs = np.ascontiguousarray(np.asarray(dvecs, dtype=np.float32))
    in_maps = [
        {"dvecs": dvecs[c * LOCAL:(c + 1) * LOCAL]} for c in range(N_CORES)
    ]
    res = run_bass_kernel_spmd(nc, in_maps, core_ids=list(range(N_CORES)))
    total = 0.0
    for c in range(N_CORES):
        o = np.asarray(res.results[c]["out"], dtype=np.float64)
        s, diag = o[0], o[1]
        rows = np.log(s) - w_val * np.maximum(diag, EPS)
        total += float(rows.sum())
    return np.float32(total / N)



# revision 3
# speedup vs baseline: 1.0081x; 1.0081x over previous
"""AngProtoLoss (stable) distributed Bass kernel for 8 TRN2 NeuronCores, v2.

Column-block scheme, NO device collectives:
  - Each core owns 512 speakers (columns k of the NxN cos matrix) and
    computes e[k, i] = exp(w*cos_ik) for ALL 4096 rows i.
  - The full u matrix (last utterance of every speaker) is shipped to every
    core from the host, pre-transposed to [d, i], pre-scaled by 4w/|u_i|
    per column, cast to fp8 (host marshaling, like the baseline's shard
    slicing).  Columns are rotated by 512*c per core so the diagonal block
    always lands at i' = k_local: a single SPMD program works for all cores.
  - Per chunk of 128 speakers: DVE binary-tree centroid sum in bf16,
    rsqrt via Ln+Exp (one ACT table set, zero switches), PE transpose ->
    fp8 cT (stationary), fp8x fp8 DoubleRow matmuls against uT (moving),
    ACT exp(S/16) from PSUM, DVE max(e,1) [exp(w*max(cos,eps)) ==
    max(exp(w*cos),1) up to 1e-5 rel], then a ones-vector matmul reduces
    over the chunk's 128 partitions into per-i partial sums accumulated
    across chunks in PSUM.
  - Outputs per core: 4096 partial exp-sums + 512 diagonal e_kk values.
    Host: s_i = sum over cores (after unrotating), cos_ii = log(e_ii)/w,
    loss = mean(log s_i - w*max(cos_ii, eps)).  (b cancels exactly.)
"""

import os
import sys

for _p in ("/opt/trn_rl_repo",):
    if os.path.isdir(_p) and _p not in sys.path:
        sys.path.append(_p)

import math

import numpy as np
import ml_dtypes

import concourse.bass as bass
import concourse.tile as tile
from concourse import bacc, mybir
from concourse.bass_utils import run_bass_kernel_spmd
from concourse.masks import make_identity

N_CORES = 8
N, M, D = 4096, 16, 512
P = 128
LOCAL = N // N_CORES        # 512 speaker columns per core
NCHUNK = LOCAL // P         # 4 chunks of 128 columns
NT = D // P                 # 4 d-subtiles of 128
NB = N // 512               # 8 i-bites of 512 columns of the moving tensor
EPS = 1e-6
UT_BOOST = 4.0              # folded into host uT scaling
CT_BOOST = 4.0              # folded into the rsqrt bias (ln 4)
EXP_SCALE = 1.0 / (UT_BOOST * CT_BOOST)

F32 = mybir.dt.float32
BF16 = mybir.dt.bfloat16
FP8 = mybir.dt.float8e4
AF = mybir.ActivationFunctionType
DR = mybir.MatmulPerfMode.DoubleRow


def build_program():
    nc = bacc.Bacc("TRN2", target_bir_lowering=False, debug=False,
                   num_devices=N_CORES)
    x = nc.dram_tensor("x", [LOCAL, M, D], FP8, kind="ExternalInput").ap()
    ut = nc.dram_tensor("ut", [D, N], FP8, kind="ExternalInput").ap()
    out = nc.dram_tensor("out", [9, 512], F32, kind="ExternalOutput").ap()

    with tile.TileContext(nc) as tc:
        _pin_act_table(nc)
        _build(nc, tc, x, ut, out)
    nc.compile()
    return nc


def _pin_act_table(nc):
    """Load the ln+exp table once up front so the compile-time table pass
    never has to thrash between natural_log and exp_and_others (each load
    is a ~1.3us TDRAM DMA)."""
    from concourse.hw_specs import get_activation_tables
    tables = list(get_activation_tables(nc.m.arch).keys())
    tid = tables.index("natural_log_exp_and_others")
    nc.scalar.add_instruction(mybir.InstLoadActFuncSet(
        name=nc.get_next_instruction_name(), ins=[], outs=[],
        act_func_set_id=tid))


def _build(nc, tc, x, ut, out):
    from contextlib import ExitStack
    ctx = ExitStack()
    with ctx:
        singles = ctx.enter_context(tc.tile_pool(name="singles", bufs=1))
        xpool = ctx.enter_context(tc.tile_pool(name="xpool", bufs=4))
        cpool = ctx.enter_context(tc.tile_pool(name="cpool", bufs=2))
        ctpool = ctx.enter_context(tc.tile_pool(name="ctpool", bufs=2))
        stats = ctx.enter_context(tc.tile_pool(name="stats", bufs=4))
        epool = ctx.enter_context(tc.tile_pool(name="epool", bufs=8))
        empool = ctx.enter_context(tc.tile_pool(name="empool", bufs=5))
        wpsum = ctx.enter_context(tc.tile_pool(name="wpsum", bufs=1, space="PSUM"))
        mpsum = ctx.enter_context(tc.tile_pool(name="mpsum", bufs=2, space="PSUM"))
        spsum = ctx.enter_context(tc.tile_pool(name="spsum", bufs=1, space="PSUM"))

        # identity (bf16): rhs of the transpose matmuls + diag-extract mask
        ident = singles.tile([P, P], BF16)
        make_identity(nc, ident)
        # 0.25-identity, fp8, in DoubleRowSwInterleave weight layout:
        # free position f = 2*(127-j)+s holds the (ksub=s, col j) weight,
        # i.e. nonzero iff 2*k + f - 254 - s == 0.  out[j, f] =
        # 0.25*(x[j,2m] + x[j,2m+1]) accumulated over m in PSUM.
        identq2 = singles.tile([P, 2, P], FP8)
        nc.gpsimd.memset(identq2, 0.0)
        iq2v = identq2.rearrange("p a b -> p (a b)")
        for s in range(2):
            nc.gpsimd.affine_select(
                out=iq2v, in_=iq2v,
                compare_op=mybir.AluOpType.not_equal,
                fill=0.25, base=-254 - s, pattern=[[1, 2 * P]],
                channel_multiplier=2)
        ones = singles.tile([P, 1], BF16)
        nc.vector.memset(ones, 1.0)
        e_diag = singles.tile([P, NCHUNK], F32)

        ut_sb = singles.tile([P, NT, N], FP8)       # [d%128, d//128, i]
        # persistent per-i partial sums: slot for bite b lives in tile b//4
        # at partition base 32*(b%4) (explicit tile_position allows base 96)
        sum_ps = [spsum.tile([P, 512], F32, name=f"sps{j}") for j in range(2)]

        # ---------- loads, all on the sync ring (the only DGE ring that
        # spreads across all 16 DMA queues): x0, then uT (needed by the
        # first main matmul, ~when tree(0)+transpose(0) finish), then the
        # remaining chunks ----------
        xs = []
        for r in range(NCHUNK):
            xr = xpool.tile([P, M, D], FP8, name=f"x{r}", tag="x")
            # two m-halves per chunk so the first tree matmuls can start
            # while the second half is still in flight
            nc.sync.dma_start(out=xr[:, :M // 2, :],
                              in_=x[r * P:(r + 1) * P, :M // 2, :])
            nc.sync.dma_start(out=xr[:, M // 2:, :],
                              in_=x[r * P:(r + 1) * P, M // 2:, :])
            xs.append(xr)
            if r == 0:
                nc.sync.dma_start(
                    out=ut_sb, in_=ut.rearrange("(t p) i -> p t i", p=P))

        # Per-chunk work, software-pipelined: the epilogue of chunk r-1
        # (emax / diag extraction on DVE, partition-sum matmuls on PE) is
        # emitted during iteration r so it never head-of-line blocks the
        # next chunk's tree / transpose / main matmuls in the engine FIFOs.
        e_tiles = [None] * NCHUNK    # per chunk: [e pair tiles]
        em_tiles = [None] * NCHUNK   # per chunk: [(pair, emax tile)]
        rs_tiles = [None] * NCHUNK   # per chunk: 1/|csum| scale AP



        def epilogue(r):
            em_tiles[r] = []
            for pj, e in enumerate(e_tiles[r]):
                if pj == 0:
                    dscr = stats.tile([P, P], BF16, name=f"dg{r}", tag="dg")
                    nc.vector.tensor_mul(
                        dscr, e[:, r * P:(r + 1) * P], ident)
                    nc.vector.tensor_reduce(
                        e_diag[:, r:r + 1], dscr,
                        axis=mybir.AxisListType.X, op=mybir.AluOpType.add)
                em = empool.tile([P, 2 * 512], BF16, name=f"em{r}_{pj}",
                                 tag="em")
                nc.vector.tensor_scalar_max(em, e, 1.0)
                em_tiles[r].append((pj, em))

        for r in range(NCHUNK):
            xr = xs[r]
            # ---- centroid sum on the PE: 8 DoubleRow matmuls against the
            # static 0.25-identity, accumulating 0.25*sum_m x in PSUM ----
            cps = wpsum.tile([P, D], F32, name=f"cps{r}", tag="cps")
            for m2 in range(M // 2):
                nc.tensor.matmul(cps, identq2, xr[:, 2 * m2:2 * m2 + 2, :],
                                 start=(m2 == 0), stop=(m2 == M // 2 - 1),
                                 perf_mode=mybir.MatmulPerfMode.DoubleRowSwInterleave)
            csum = cpool.tile([P, D], BF16, name=f"csum{r}", tag="csum")
            nc.vector.tensor_copy(csum, cps)

            # previous chunk's DVE epilogue
            if r >= 1:
                epilogue(r - 1)

            # ---- rs = 1/(4*|csum|) = exp(-0.5*ln(16*ssq)); the 4 cancels
            # the host-side 4w/|u| boost times the 0.25 in identq2 ----
            sq_scr = cpool.tile([P, D], BF16, name=f"sq{r}", tag="sq")
            ssq = stats.tile([P, 1], F32, name=f"ssq{r}", tag="ssq")
            nc.vector.scalar_tensor_tensor(
                out=sq_scr, in0=csum, scalar=1.0, in1=csum,
                op0=mybir.AluOpType.mult, op1=mybir.AluOpType.mult,
                accum_out=ssq)
            lnv = stats.tile([P, 1], F32, name=f"ln{r}", tag="ln")
            nc.scalar.activation(lnv, ssq, AF.Ln, scale=16.0)
            rs = stats.tile([P, 1], F32, name=f"rs{r}", tag="rs")
            nc.scalar.activation(rs, lnv, AF.Exp, scale=-0.5)
            rs_tiles[r] = rs

            # ---- transpose via matmul against the identity ----
            # all 4 transposes share one PSUM bank at different offsets
            cT = ctpool.tile([P, NT, P], FP8, name=f"cT{r}", tag="cT")
            pt = wpsum.tile([P, NT, P], F32, name=f"pt{r}", tag="pt")
            for t in range(NT):
                nc.tensor.matmul(pt[:, t, :], csum[:, t * P:(t + 1) * P],
                                 ident, start=True, stop=True)
            for h in range(2):
                nc.vector.tensor_copy(cT[:, 2 * h:2 * h + 2, :],
                                      pt[:, 2 * h:2 * h + 2, :])

            # ---- main matmuls + exp, two pairs per wave ----
            e_tiles[r] = []
            for w0 in range(2):            # wave: pairs (2*w0, 2*w0+1)
                pss = []
                for pj in (2 * w0, 2 * w0 + 1):
                    ps = mpsum.tile([P, 2, 512], F32, name=f"ps{r}_{pj}",
                                    tag="ps")
                    pss.append(ps)
                for h in range(2):
                    for pi, pj in enumerate((2 * w0, 2 * w0 + 1)):
                        for b in range(2):
                            bite = 2 * pj + b
                            nc.tensor.matmul(
                                pss[pi][:, b, :],
                                cT[:, 2 * h:2 * h + 2, :],
                                ut_sb[:, 2 * h:2 * h + 2,
                                      bite * 512:(bite + 1) * 512],
                                start=(h == 0), stop=(h == 1),
                                perf_mode=DR)
                for pi, pj in enumerate((2 * w0, 2 * w0 + 1)):
                    e = epool.tile([P, 2 * 512], BF16, name=f"e{r}_{pj}",
                                   tag="e")
                    nc.scalar.activation(
                        e, pss[pi].rearrange("p a b -> p (a b)"),
                        AF.Exp, scale=rs[:, 0:1])
                    e_tiles[r].append(e)

            # previous chunk's partition-sum matmuls, after this chunk's
            # main matmuls on the PE FIFO
            if r >= 1:
                _sum_mms(nc, sum_ps, ones, em_tiles[r - 1], r - 1)

        # ---- tail: last chunk's epilogue + outputs ----
        epilogue(NCHUNK - 1)
        nc.sync.dma_start(out=out[8].rearrange("(r p) -> p r", p=P),
                          in_=e_diag)
        _sum_mms(nc, sum_ps, ones, em_tiles[NCHUNK - 1], NCHUNK - 1)
        s_sb = [singles.tile([P, 512], F32, name=f"ssb{j}") for j in range(2)]
        for j in range(2):
            nc.vector.tensor_copy(s_sb[j], sum_ps[j])
        for bite in range(NB):
            j, s = bite // 4, bite % 4
            nc.sync.dma_start(out=out[bite],
                              in_=s_sb[j][32 * s:32 * s + 1, :])


def _sum_mms(nc, sum_ps, ones, em_list, r):
    for pj, em in em_list:
        for b in range(2):
            bite = 2 * pj + b
            j, s = bite // 4, bite % 4
            nc.tensor.matmul(
                sum_ps[j][32 * s:32 * s + 1, :],
                ones,
                em[:, b * 512:(b + 1) * 512],
                start=(r == 0), stop=(r == NCHUNK - 1),
                tile_position=(0, 32 * s))


_CACHE = {}


def _get_program():
    if "nc" not in _CACHE:
        _CACHE["nc"] = build_program()
    return _CACHE["nc"]


def _prep_inputs(dvecs, w_val):
    dv = np.asarray(dvecs, dtype=np.float32)
    x8 = dv.astype(ml_dtypes.float8_e4m3)                   # (N, M, D)
    u = dv[:, M - 1, :].astype(np.float64)                  # (N, D)
    unorm = np.sqrt((u * u).sum(axis=1))                    # (N,)
    scale = (UT_BOOST * w_val) / unorm                      # (N,)
    utw = (u * scale[:, None]).T.astype(np.float32)         # (D, N)
    ut8 = utw.astype(ml_dtypes.float8_e4m3)
    in_maps = []
    for c in range(N_CORES):
        in_maps.append({
            "x": np.ascontiguousarray(x8[c * LOCAL:(c + 1) * LOCAL]),
            "ut": np.ascontiguousarray(np.roll(ut8, -LOCAL * c, axis=1)),
        })
    return in_maps


def kernel(dvecs, w, b):
    w_val = float(np.asarray(w))
    nc = _get_program()
    in_maps = _prep_inputs(dvecs, w_val)
    res = run_bass_kernel_spmd(nc, in_maps, core_ids=list(range(N_CORES)))
    s_tot = np.zeros(N, dtype=np.float64)
    diag_e = np.zeros(N, dtype=np.float64)
    for c in range(N_CORES):
        o = np.asarray(res.results[c]["out"], dtype=np.float64)
        s_tot += np.roll(o[:8].reshape(N), LOCAL * c)
        diag_e[c * LOCAL:(c + 1) * LOCAL] = o[8]
    cos_d = np.log(np.maximum(diag_e, 1e-300)) / w_val
    rows = np.log(s_tot) - w_val * np.maximum(cos_d, EPS)
    return np.float32(rows.mean())
